# revision 42
# baseline (speedup 1.0000x reference)
"""Trainium2 Bass kernel for nn_CROSSLoss (softmax-entropy * mean-cosine-similarity loss).

Math (reference):
    logits = x @ W + b                       [B, C]
    loss_i = sum_n softmax(logits)_in * log_softmax(logits)_in
    xn     = x / max(||x_i||, eps)
    weight_i = (1/B) * sum_j xn_i . xn_j
    out_i  = loss_i * weight_i

Key restructurings:
  * weight_i = xn_i . s / B with s = sum_j xn_j -- the BxB similarity matrix
    is never materialized; the per-core partial of the [D] vector s is the
    only cross-core data. It moves via ONE AllGather (4KB/core) + local tree
    reduce: one ring pass instead of an AllReduce's two.
  * logits are small (|l| < 4), so softmax needs no max subtraction:
    loss = S2/Z - ln Z with Z = sum e^l and S2 = sum l*e^l.
  * S2 options: "eps" computes S2/Z by central difference from two ACT
    exp-accum passes (za = sum e^{(1+eps)l}, zb = sum e^{(1-eps)l};
    S2/Z = (za-zb)/(eps(za+zb)), O(eps^2) bias ~1e-5); "dve" uses the
    direct exp + multiply + reduce.
  * The logits GEMM runs in fp8 (e4m3) with MatmulPerfMode.DoubleRow: two
    128-deep contraction tiles per pass, 2x PE throughput. W is pre-scaled
    by 256 on the host into e4m3's normal range; the 1/256 rides in the
    exp scale and the final loss arithmetic.
  * Schedule shaping: chunk-0 logits warm the PE, then the rank-1 s matmuls
    (collective input) preempt the remaining logits via a scheduler hint;
    dummy rank-1 matmuls keep PE clocks up across the collective so the
    u matmuls run at full p-state.
  * Tail stays in row space: t1 = loss*r/B transposes through DRAM during
    the collective; the final multiply reads u straight from PSUM.

Sharding: data-parallel over batch; 1024 rows/core. Per core loads: x rows
f16 (norm/s path), xt f16 (u matmul), xt fp8 + W fp8 (logits), 6MB total.
"""

import numpy as np

N_CORES = 8
B, D, C = 8192, 1024, 1000
B_LOC = B // N_CORES  # rows per core
P = 128
RC = B_LOC // P  # row chunks per core
KC = D // P  # contraction chunks
N0 = 512
N1 = C - N0
W_SCALE = 256.0  # host-side W pre-scale for fp8 range
R_SCALE = 64.0  # host-independent r pre-scale for fp8 s matmul
S2_EPS = 0.01  # central-difference step for S2
# linear seed y0 = C1/ss + C0 for rsqrt over ss in [810, 1240] (2.8e-3 max
# rel; one Newton iteration brings it to 1.2e-5)
RSQRT_C1 = 15.78414098
RSQRT_C0 = 0.01574952754

_CACHE = {}

# knobs for A/B experiments
COLLECTIVE = "allreduce"  # "allreduce" | "allgather" (HW: same ~2.3us, AR has shorter tail)
FP8 = True  # fp8 DoubleRow logits matmul
X8 = False  # phase A on fp8 x: rejected, sqrt(B)-amplified s noise (2.6% weight err)
S2_MODE = "dve"  # "dve" (exp + mul + reduce) | "eps" (2 exp passes)
DVE_SS = (1, 3, 5)  # phase-A chunks squared on DVE instead of ACT
NEWTON_ITERS = 1
R_MODE = "lnexp"  # "lnexp" (ACT exp(-.5 ln ss)) | "newton"
PE_FILLER = 0  # dummy rank-1 matmul pairs keeping PE warm pre-u (HW: hurts)
LOGITS_HINT_US = 0.0  # scheduler hint: delay logits c1.. past s matmuls (HW: off)


def _build(
    with_bias: bool,
    repeat: int = 1,
    collective: str = COLLECTIVE,
    fp8: bool = FP8,
    x8: bool = X8,
    s2_mode: str = S2_MODE,
    dve_ss: tuple = DVE_SS,
    newton_iters: int = NEWTON_ITERS,
    r_mode: str = R_MODE,
    pe_filler: int = PE_FILLER,
    logits_hint_us: float = LOGITS_HINT_US,
    debug: bool = False,
    hw_loop: int = 0,
    chain: bool = False,
    timing_build: bool = False,
    skip: tuple = (),
):
    from contextlib import ExitStack

    import concourse.bacc as bacc
    import concourse.tile as tile
    from concourse import mybir

    f8 = mybir.dt.float8e4
    bf16 = mybir.dt.bfloat16
    f16 = mybir.dt.float16
    f32 = mybir.dt.float32
    Alu = mybir.AluOpType
    Act = mybir.ActivationFunctionType
    DR = mybir.MatmulPerfMode.DoubleRow

    nc = bacc.Bacc(None, num_devices=N_CORES)

    x_dt = f8 if x8 else f16
    if timing_build:
        # timing-only: big inputs live in internal DRAM (uninitialized), so
        # per-call dispatch ships no data and wall-clock noise stays tiny
        x_h = nc.dram_tensor("x_t", [B_LOC, D], x_dt)
        xt_h = nc.dram_tensor("xt_t", [D, B_LOC], f16)
        if fp8:
            xt8_h = nc.dram_tensor("xt8_t", [D, B_LOC], f8)
            w_h = nc.dram_tensor("w8_t", [D, C], f8)
        else:
            w_h = nc.dram_tensor("w_t", [D, C], f16)
    else:
        x_h = nc.declare_dram_parameter(
            "x8_h" if x8 else "x_h", [B_LOC, D], x_dt, isOutput=False
        )
        xt_h = nc.declare_dram_parameter("xt_h", [D, B_LOC], f16, isOutput=False)
        if fp8:
            xt8_h = nc.declare_dram_parameter("xt8_h", [D, B_LOC], f8, isOutput=False)
            w_h = nc.declare_dram_parameter("w8_h", [D, C], f8, isOutput=False)
        else:
            w_h = nc.declare_dram_parameter("w_h", [D, C], f16, isOutput=False)
    b_h = (
        nc.declare_dram_parameter("b_h", [1, C], f16, isOutput=False)
        if with_bias
        else None
    )
    out_f = nc.declare_dram_parameter("out_f", [1, B_LOC], f32, isOutput=True)
    if debug:
        dbg_za = nc.declare_dram_parameter("dbg_za", [P, RC], f32, isOutput=True)
        dbg_zb = nc.declare_dram_parameter("dbg_zb", [P, RC], f32, isOutput=True)
        dbg_r = nc.declare_dram_parameter("dbg_r", [P, RC], f32, isOutput=True)
        dbg_s = nc.declare_dram_parameter("dbg_s", [P, KC], f32, isOutput=True)
        dbg_u = nc.declare_dram_parameter("dbg_u", [1, B_LOC], f32, isOutput=True)
        dbg_u2 = nc.declare_dram_parameter("dbg_u2", [1, B_LOC], f32, isOutput=True)
        dbg_u3 = nc.declare_dram_parameter("dbg_u3", [1, C], f32, isOutput=True)
        dbg_t1 = nc.declare_dram_parameter("dbg_t1", [1, B_LOC], f32, isOutput=True)

    cc_dt = f16
    # double-buffered across repeat iterations so iteration i+1's collective
    # input write never waits on iteration i's collective completion
    cc_ins = [nc.dram_tensor(f"cc_in{j}", [1, D], cc_dt) for j in range(2)]
    if collective == "allgather":
        cc_outs = [
            nc.dram_tensor(f"cc_out{j}", [N_CORES, D], cc_dt, addr_space="Shared")
            for j in range(2)
        ]
    else:
        cc_outs = [
            nc.dram_tensor(f"cc_out{j}", [1, D], cc_dt, addr_space="Shared")
            for j in range(2)
        ]
    t1_drams = [nc.dram_tensor(f"t1_dram{j}", [1, B_LOC], f32) for j in range(2)]

    w_dt = f8 if fp8 else f16
    l_scale = (1.0 / W_SCALE) if fp8 else 1.0

    with tile.TileContext(nc) as tc, ExitStack() as ctx:
        singles = ctx.enter_context(tc.tile_pool(name="singles", bufs=1))
        # big per-iteration inputs double-buffer so iteration i+1's DMAs and
        # logits overlap iteration i's collective bubble
        bigs = ctx.enter_context(tc.tile_pool(name="bigs", bufs=2))
        work = ctx.enter_context(tc.tile_pool(name="work", bufs=3))
        lps = ctx.enter_context(tc.tile_pool(name="lps", bufs=2, space="PSUM"))
        vps = ctx.enter_context(tc.tile_pool(name="vps", bufs=1, space="PSUM"))

        if with_bias:
            b_sb = singles.tile([1, C], f16)
            ones = singles.tile([1, P], f16)

        # newton-path + eps-path temporaries (unused by default)
        rs_h = singles.tile([P, RC], f32)
        rs_a = singles.tile([P, RC], f32)
        rs_b = singles.tile([P, RC], f32)
        zs_all = singles.tile([P, RC], f32)
        zd_all = singles.tile([P, RC], f32)
        # small per-iteration stat tiles rotate (bufs=2) so iteration i+1's
        # phase A never waits on iteration i's loss-math readers
        stats = ctx.enter_context(tc.tile_pool(name="stats", bufs=2))

        def stat_tiles():
            ss_all = stats.tile([P, RC], f32, tag="ss", name="ss_all")
            rs_g = stats.tile([P, RC], f32, tag="rsg", name="rs_g")
            r_all = stats.tile([P, RC], f32, tag="r", name="r_all")
            r16 = stats.tile([P, RC], f16, tag="r16", name="r16")
            za_all = stats.tile([P, RC], f32, tag="za", name="za_all")
            zb_all = stats.tile([P, RC], f32, tag="zb", name="zb_all")
            lnz = stats.tile([P, RC], f32, tag="lnz", name="lnz")
            rz = stats.tile([P, RC], f32, tag="rz", name="rz")
            s2z = stats.tile([P, RC], f32, tag="s2z", name="s2z")
            loss = stats.tile([P, RC], f32, tag="loss", name="loss")
            t1 = stats.tile([P, RC], f32, tag="t1", name="t1")
            t1_row = stats.tile([1, B_LOC], f32, tag="t1r", name="t1_row")
            out_row = stats.tile([1, B_LOC], f32, tag="or", name="out_row")
            s16 = stats.tile([P, KC], f16, tag="s16", name="s16")
            s_row = stats.tile([1, D], f16, tag="srow", name="s_row")
            return (ss_all, rs_g, r_all, r16, za_all, zb_all, lnz, rz, s2z,
                    loss, t1, t1_row, out_row, s16, s_row)
        if collective == "allgather":
            s_g = singles.tile([P, N_CORES, KC], f16)  # [p][src core][k]
            s_t4 = singles.tile([P, 4, KC], f32)
            s_t2 = singles.tile([P, 2, KC], f32)
            s_gr = singles.tile([P, 1, KC], f32)
        if collective != "allgather":
            s_f32 = singles.tile([P, KC], f32)

        s_ps = vps.tile([1, D], f32, tag="s", name="s_ps")
        u_ps = vps.tile([1, B_LOC], f32, tag="u", name="u_ps")
        if chain:
            chain_sb = singles.tile([1, 8], f32)

        if with_bias:
            nc.vector.memset(ones, 1.0)

        def logits_chunk(c, xt8_sb, xt_sb, w_sb):
            lpsum = lps.tile([P, C], f32, tag="logits")
            last_k_stops = not with_bias
            if fp8:
                for kk in range(KC // 2):
                    lt = xt8_sb[:, 2 * kk : 2 * kk + 2, c * P : (c + 1) * P]
                    nc.tensor.matmul(
                        lpsum[:, 0:N0],
                        lhsT=lt,
                        rhs=w_sb[:, 2 * kk : 2 * kk + 2, 0:N0],
                        start=(kk == 0),
                        stop=(last_k_stops and kk == KC // 2 - 1),
                        perf_mode=DR,
                    )
                    nc.tensor.matmul(
                        lpsum[:, N0:C],
                        lhsT=lt,
                        rhs=w_sb[:, 2 * kk : 2 * kk + 2, N0:C],
                        start=(kk == 0),
                        stop=(last_k_stops and kk == KC // 2 - 1),
                        perf_mode=DR,
                    )
            else:
                for k in range(KC):
                    lt = xt_sb[:, k, c * P : (c + 1) * P]
                    nc.tensor.matmul(
                        lpsum[:, 0:N0],
                        lhsT=lt,
                        rhs=w_sb[:, k, 0:N0],
                        start=(k == 0),
                        stop=(last_k_stops and k == KC - 1),
                    )
                    nc.tensor.matmul(
                        lpsum[:, N0:C],
                        lhsT=lt,
                        rhs=w_sb[:, k, N0:C],
                        start=(k == 0),
                        stop=(last_k_stops and k == KC - 1),
                    )
            if with_bias:
                nc.tensor.matmul(
                    lpsum[:, 0:N0],
                    lhsT=ones,
                    rhs=b_sb[:, 0:N0],
                    start=False,
                    stop=True,
                    skip_group_check=True,
                )
                nc.tensor.matmul(
                    lpsum[:, N0:C],
                    lhsT=ones,
                    rhs=b_sb[:, N0:C],
                    start=False,
                    stop=True,
                    skip_group_check=True,
                )
            return lpsum

        def stats_chunk(c, lpsum):
            if s2_mode == "eps":
                ea = work.tile([P, C], bf16, tag="ea")
                nc.scalar.activation(
                    out=ea,
                    in_=lpsum,
                    func=Act.Exp,
                    scale=l_scale * (1.0 + S2_EPS),
                    accum_out=za_all[:, c : c + 1],
                )
                nc.scalar.activation(
                    out=ea,
                    in_=lpsum,
                    func=Act.Exp,
                    scale=l_scale * (1.0 - S2_EPS),
                    accum_out=zb_all[:, c : c + 1],
                )
            else:
                e_t = work.tile([P, C], bf16, tag="e")
                nc.scalar.activation(
                    out=e_t,
                    in_=lpsum,
                    func=Act.Exp,
                    scale=l_scale,
                    accum_out=za_all[:, c : c + 1],
                )
                prod = work.tile([P, C], bf16, tag="prod")
                nc.vector.scalar_tensor_tensor(
                    out=prod,
                    in0=lpsum,
                    scalar=1.0,
                    in1=e_t,
                    op0=Alu.mult,
                    op1=Alu.mult,
                    accum_out=zb_all[:, c : c + 1],
                )

        from contextlib import nullcontext

        loop_cm = tc.For_i(0, hw_loop) if hw_loop else nullcontext(0)
        pending_tail = None
        with loop_cm:
          for it in range(repeat):
              cc_in = cc_ins[it % 2]
              cc_out = cc_outs[it % 2]
              t1_dram = t1_drams[it % 2]
              (ss_all, rs_g, r_all, r16, za_all, zb_all, lnz, rz, s2z,
               loss, t1, t1_row, out_row, s16, s_row) = stat_tiles()
              # rotating per-iteration input tiles
              x_sb = bigs.tile([P, RC, D], x_dt, tag="x", name="x_sb")
              xt_sb = bigs.tile([P, KC, B_LOC], f16, tag="xt", name="xt_sb")
              xt8_sb = (
                  bigs.tile([P, KC, B_LOC], f8, tag="xt8", name="xt8_sb")
                  if fp8
                  else None
              )
              w_sb = bigs.tile([P, KC, C], w_dt, tag="w", name="w_sb")
              # ---- input DMAs ----
              # ALL bulk inputs ride the SP HWDGE queue: x first (it gates the
              # latency-critical chain ss -> r -> s -> collective), then the
              # fp8 logits operands, then xt16 last (only needed post-collective
              # for the u matmul). The gpsimd SWDGE queue is reserved for the
              # collective-adjacent small DMAs so the input stream never queues
              # behind a collective-gated wait (engine DMA FIFOs are in-order).
              need_x = (not ({"a", "s"} <= set(skip))) or pe_filler > 0
              for c in (range(RC) if need_x else ()):
                  nc.sync.dma_start(out=x_sb[:, c, :], in_=x_h[c * P : (c + 1) * P, :])
              # NB: a [:, k:k+2, :] SBUF destination iterates [p][j][i], so the
              # DRAM side must supply rows in (p j) order -- a plain [256, D]
              # slice would interleave row pairs.
              for k in ([] if "c" in skip else range(0, KC, 2)):
                  nc.sync.dma_start(
                      out=w_sb[:, k : k + 2, :],
                      in_=w_h[k * P : (k + 2) * P, :].rearrange("(j p) i -> p j i", j=2),
                  )
                  if fp8:
                      nc.sync.dma_start(
                          out=xt8_sb[:, k : k + 2, :],
                          in_=xt8_h[k * P : (k + 2) * P, :].rearrange(
                              "(j p) i -> p j i", j=2
                          ),
                      )
              for k in ([] if "u" in skip else range(0, KC, 2)):
                  nc.sync.dma_start(
                      out=xt_sb[:, k : k + 2, :],
                      in_=xt_h[k * P : (k + 2) * P, :].rearrange("(j p) i -> p j i", j=2),
                  )
              if with_bias:
                  nc.sync.dma_start(out=b_sb, in_=b_h[:, :])

              # ---- Phase A: per-chunk pipeline square -> r -> s matmul so
              # the collective input is ready as soon as the last x chunk
              # lands (r per chunk instead of one [P, RC] batch)
              if "a" in skip:
                  nc.vector.memset(r16, 0.01)
                  nc.vector.memset(r_all, 0.01)
              for c in ([] if "a" in skip else range(RC)):
                  sq = work.tile([P, D], f16, tag="sq")
                  if c not in dve_ss:
                      nc.scalar.activation(
                          out=sq,
                          in_=x_sb[:, c, :],
                          func=Act.Square,
                          accum_out=ss_all[:, c : c + 1],
                      )
                  else:
                      nc.vector.scalar_tensor_tensor(
                          out=sq,
                          in0=x_sb[:, c, :],
                          scalar=1.0,
                          in1=x_sb[:, c, :],
                          op0=Alu.mult,
                          op1=Alu.mult,
                          accum_out=ss_all[:, c : c + 1],
                      )
                  if r_mode == "lnexp":
                      nc.scalar.activation(
                          out=rs_g[:, c : c + 1],
                          in_=ss_all[:, c : c + 1],
                          func=Act.Ln,
                      )
                      nc.scalar.activation(
                          out=r_all[:, c : c + 1],
                          in_=rs_g[:, c : c + 1],
                          func=Act.Exp,
                          scale=-0.5,
                      )
                      nc.vector.tensor_copy(
                          out=r16[:, c : c + 1], in_=r_all[:, c : c + 1]
                      )
                  if "s" not in skip:
                      nc.tensor.matmul(
                          s_ps[:, 0:512],
                          lhsT=r16[:, c : c + 1],
                          rhs=x_sb[:, c, 0:512],
                          start=(c == 0),
                          stop=(c == RC - 1),
                      )
                      nc.tensor.matmul(
                          s_ps[:, 512:1024],
                          lhsT=r16[:, c : c + 1],
                          rhs=x_sb[:, c, 512:1024],
                          start=(c == 0),
                          stop=(c == RC - 1),
                      )
              # newton fallback (unused by default)
              if "a" not in skip and r_mode != "lnexp":
               nc.vector.tensor_scalar_mul(out=rs_h, in0=ss_all, scalar1=0.5)
               nc.vector.reciprocal(out=rs_g, in_=ss_all)
               nc.vector.tensor_scalar(
                   out=r_all,
                   in0=rs_g,
                   scalar1=RSQRT_C1,
                   scalar2=RSQRT_C0,
                   op0=Alu.mult,
                   op1=Alu.add,
               )
               for _i in (range(newton_iters) if r_mode != "lnexp" else []):
                   nc.vector.tensor_tensor(out=rs_a, in0=r_all, in1=r_all, op=Alu.mult)
                   nc.vector.tensor_tensor(out=rs_b, in0=rs_h, in1=rs_a, op=Alu.mult)
                   nc.vector.tensor_scalar(
                       out=rs_b,
                       in0=rs_b,
                       scalar1=-1.0,
                       scalar2=1.5,
                       op0=Alu.mult,
                       op1=Alu.add,
                   )
                   nc.vector.tensor_tensor(out=r_all, in0=r_all, in1=rs_b, op=Alu.mult)
               if r_mode != "lnexp":
                   nc.vector.tensor_copy(out=r16, in_=r_all)

              # ---- Phase B: share s across the 8 cores ----
              # s_row holds s in d=(k p) order; cc_in is written (p k)-transposed
              # so the post-collective gather reads 32B-contiguous runs.
              if "s" in skip:
                  nc.vector.memset(s16, 0.01)
              elif True:
                  nc.vector.tensor_copy(out=s_row, in_=s_ps)
              if "s" in skip:
                  pass
              elif collective == "allgather":
                  nc.gpsimd.dma_start(
                      out=cc_in[0, :].rearrange("(p k) -> k p", k=KC),
                      in_=s_row[:, :],
                  )
                  nc.gpsimd.collective_compute(
                      "AllGather",
                      mybir.AluOpType.bypass,
                      replica_groups=[list(range(N_CORES))],
                      ins=[cc_in[:, :]],
                      outs=[cc_out[:, :]],
                  )
                  nc.gpsimd.dma_start(
                      out=s_g[:, :, :],
                      in_=cc_out[:, :].rearrange("c (p k) -> p c k", k=KC),
                  )
                  nc.vector.tensor_tensor(
                      out=s_t4, in0=s_g[:, 0:4, :], in1=s_g[:, 4:8, :], op=Alu.add
                  )
                  nc.vector.tensor_tensor(
                      out=s_t2, in0=s_t4[:, 0:2, :], in1=s_t4[:, 2:4, :], op=Alu.add
                  )
                  nc.vector.tensor_tensor(
                      out=s_gr, in0=s_t2[:, 0:1, :], in1=s_t2[:, 1:2, :], op=Alu.add
                  )
                  nc.vector.tensor_copy(out=s16, in_=s_gr[:, 0, :])
              elif collective == "none":
                  # timing probe only: s = own partial (wrong result)
                  nc.gpsimd.dma_start(out=cc_in[:, :], in_=s_row[:, :])
              else:
                  nc.gpsimd.dma_start(out=cc_in[:, :], in_=s_row[:, :])
                  nc.gpsimd.collective_compute(
                      "AllReduce",
                      mybir.AluOpType.add,
                      replica_groups=[list(range(N_CORES))],
                      ins=[cc_in[:, :]],
                      outs=[cc_out[:, :]],
                  )

              # ---- Phase C: remaining logits + stats ----
              if "c" in skip:
                  nc.vector.memset(za_all, 1000.0)
                  nc.vector.memset(zb_all, 999.0)
              else:
                  for c in range(RC):
                      lpsum = logits_chunk(c, xt8_sb, xt_sb, w_sb)
                      stats_chunk(c, lpsum)

              # loss math (column layout [P, RC])
              if "l" in skip:
                  nc.vector.memset(t1_row, 0.5)
                  nc.vector.memset(out_row, 0.5)
              elif s2_mode == "eps":
                  # Z = (za+zb)/2, S2/Z = (za-zb)/(eps*(za+zb))
                  nc.vector.tensor_tensor(out=zs_all, in0=za_all, in1=zb_all, op=Alu.add)
                  nc.vector.tensor_tensor(
                      out=zd_all, in0=za_all, in1=zb_all, op=Alu.subtract
                  )
                  nc.scalar.activation(out=lnz, in_=zs_all, func=Act.Ln, scale=0.5)
                  nc.vector.reciprocal(out=rz, in_=zs_all)
                  nc.vector.tensor_tensor(out=s2z, in0=zd_all, in1=rz, op=Alu.mult)
                  nc.vector.scalar_tensor_tensor(
                      out=loss,
                      in0=s2z,
                      scalar=1.0 / S2_EPS,
                      in1=lnz,
                      op0=Alu.mult,
                      op1=Alu.subtract,
                  )
              elif True:
                  # za = Z, zb = S2' = W_SCALE * S2.
                  # ln Z on DVE as ln(1280) + ln1p(y), y = Z/1280 - 1 (|y|<0.15
                  # for these inputs; quartic error ~1e-5). Avoids the ACT Ln
                  # table swap and keeps the whole tail on one engine.
                  nc.scalar.activation(out=lnz, in_=za_all, func=Act.Ln)
                  nc.vector.reciprocal(out=rz, in_=za_all)
                  nc.vector.tensor_tensor(out=s2z, in0=zb_all, in1=rz, op=Alu.mult)
                  nc.vector.scalar_tensor_tensor(
                      out=loss,
                      in0=s2z,
                      scalar=l_scale,
                      in1=lnz,
                      op0=Alu.mult,
                      op1=Alu.subtract,
                  )
              if "l" not in skip:
                  nc.vector.scalar_tensor_tensor(
                      out=t1,
                      in0=loss,
                      scalar=1.0 / B,
                      in1=r_all,
                      op0=Alu.mult,
                      op1=Alu.mult,
                  )
                  # t1 -> row layout through DRAM while the collective flies
                  nc.gpsimd.dma_start(
                      out=t1_dram[0, :].rearrange("(c p) -> p c", p=P), in_=t1[:, :]
                  )
                  nc.gpsimd.dma_start(out=t1_row[:, :], in_=t1_dram[:, :])

              # PE keep-warm filler: rank-1 matmuls on resident data, overwritten
              # by the real u accumulation group (start=True resets the banks).
              for i in range(pe_filler):
                  cc = i % RC
                  nc.tensor.matmul(
                      u_ps[:, 0:512],
                      lhsT=r16[:, cc : cc + 1],
                      rhs=x_sb[:, cc, 0:512],
                      start=(i == 0),
                      stop=(i == pe_filler - 1),
                  )
                  nc.tensor.matmul(
                      u_ps[:, 512:1024],
                      lhsT=r16[:, cc : cc + 1],
                      rhs=x_sb[:, cc, 512:1024],
                      start=(i == 0),
                      stop=(i == pe_filler - 1),
                  )

              # HAM keep-alive: tiny matmuls dep-chained on late stats/loss
              # results fire ~1-2us apart through the tail window, preventing
              # the PE idle-window detector from re-throttling to K=4/8.
              if "c" not in skip and "l" not in skip:
                  for fc in (5, 6, 7):
                      nc.tensor.matmul(
                          u_ps[0:1, 0:RC],
                          lhsT=zb_all[:, fc : fc + 1],
                          rhs=zb_all[:, 0:RC],
                          start=True,
                          stop=True,
                          skip_group_check=True,
                      )
                  nc.tensor.matmul(
                      u_ps[0:1, 0:RC],
                      lhsT=loss[:, 0:1],
                      rhs=loss[:, 0:RC],
                      start=True,
                      stop=True,
                      skip_group_check=True,
                  )
                  nc.tensor.matmul(
                      u_ps[0:1, 0:RC],
                      lhsT=t1[:, 0:1],
                      rhs=t1[:, 0:RC],
                      start=True,
                      stop=True,
                      skip_group_check=True,
                  )
                  nc.tensor.matmul(
                      u_ps[0:1, 0:RC],
                      lhsT=t1_row[0:1, 0:1],
                      rhs=t1_row[0:1, 0:RC],
                      start=True,
                      stop=True,
                      skip_group_check=True,
                  )

              # ---- Phase D (deferred tail): gather s, u = x @ s, out ----
              def make_tail(s16, xt_sb, t1_row, out_row, cc_out, cc_in):
                  def tail():
                      if "s" not in skip and collective in ("allreduce", "none"):
                          src_cc = cc_out if collective == "allreduce" else cc_in
                          nc.gpsimd.dma_start(
                              out=s16[:, 0:KC],
                              in_=src_cc[0, 0:D].rearrange("(k p) -> p k", p=P),
                          )
                      if "u" in skip:
                          nc.vector.memset(u_ps, 1.0)
                      for k in ([] if "u" in skip else range(KC)):
                          nc.tensor.matmul(
                              u_ps[:, 0:512],
                              lhsT=s16[:, k : k + 1],
                              rhs=xt_sb[:, k, 0:512],
                              start=(k == 0),
                              stop=(k == KC - 1),
                          )
                          nc.tensor.matmul(
                              u_ps[:, 512:1024],
                              lhsT=s16[:, k : k + 1],
                              rhs=xt_sb[:, k, 512:1024],
                              start=(k == 0),
                              stop=(k == KC - 1),
                          )
                      if "l" not in skip:
                          nc.vector.tensor_tensor(
                              out=out_row, in0=u_ps, in1=t1_row, op=Alu.mult
                          )
                      nc.gpsimd.dma_start(out=out_f[:, :], in_=out_row[:, :])
                  return tail

              this_tail = make_tail(s16, xt_sb, t1_row, out_row, cc_out, cc_in)
              this_tail()
              if chain:
                  # serialize repeat iterations: read the output back and
                  # scribble on x_sb so the next iteration's x DMA must wait
                  nc.sync.dma_start(out=chain_sb, in_=out_f[0:1, 0:8])
                  nc.vector.tensor_copy(out=x_sb[0:1, 0, 0:8], in_=chain_sb)

              if debug:
                  u_dbg_row = singles.tile([1, B_LOC], f32)
                  s_dbg = singles.tile([P, KC], f32)
                  nc.vector.tensor_copy(out=u_dbg_row, in_=u_ps)
                  nc.vector.tensor_copy(out=s_dbg, in_=s16)
                  # v2: weights from standalone [128,1] tiles
                  s16k = [singles.tile([P, 1], f16, name=f"s16k{k}") for k in range(KC)]
                  for k in range(KC):
                      nc.vector.tensor_copy(out=s16k[k], in_=s16[:, k : k + 1])
                  for k in range(KC):
                      nc.tensor.matmul(
                          s_ps[:, 0:512], lhsT=s16k[k], rhs=xt_sb[:, k, 0:512],
                          start=(k == 0), stop=(k == KC - 1),
                      )
                      nc.tensor.matmul(
                          s_ps[:, 512:1024], lhsT=s16k[k], rhs=xt_sb[:, k, 512:1024],
                          start=(k == 0), stop=(k == KC - 1),
                      )
                  u2_row = singles.tile([1, B_LOC], f32)
                  nc.vector.tensor_copy(out=u2_row, in_=s_ps)
                  nc.sync.dma_start(out=dbg_u2[:, :], in_=u2_row[:, :])
                  # v3: bf16 weights from a [128, KC] tile
                  s16b = singles.tile([P, KC], bf16)
                  xtb = singles.tile([P, 1024], bf16)
                  nc.vector.tensor_copy(out=s16b, in_=s16)
                  u3_ps = lps.tile([P, C], f32, tag="logits")
                  for k in range(KC):
                      nc.tensor.matmul(
                          u3_ps[0:1, 0:512], lhsT=s16b[:, k : k + 1], rhs=xt_sb[:, k, 0:512],
                          start=(k == 0), stop=(k == KC - 1), skip_group_check=True,
                      )
                      nc.tensor.matmul(
                          u3_ps[0:1, 512:1000], lhsT=s16b[:, k : k + 1], rhs=xt_sb[:, k, 512:1000],
                          start=(k == 0), stop=(k == KC - 1), skip_group_check=True,
                      )
                  u3_row = singles.tile([1, C], f32)
                  nc.vector.tensor_copy(out=u3_row, in_=u3_ps[0:1, :])
                  nc.sync.dma_start(out=dbg_u3[:, :], in_=u3_row[:, :])
                  nc.sync.dma_start(out=dbg_za[:, :], in_=za_all)
                  nc.sync.dma_start(out=dbg_zb[:, :], in_=zb_all)
                  nc.sync.dma_start(out=dbg_r[:, :], in_=r_all)
                  nc.sync.dma_start(out=dbg_s[:, :], in_=s_dbg)
                  nc.sync.dma_start(out=dbg_u[:, :], in_=u_dbg_row)
                  nc.sync.dma_start(out=dbg_t1[:, :], in_=t1_row)

    # Steer every activation onto act-func-set 6 (natural_log_exp_and_others:
    # Exp+Ln+Square+Copy in ONE table) so the kernel runs with a single
    # LoadActFuncSet and zero mid-stream table swaps. The selection pass takes
    # the first set containing each func, so present it a view with the
    # earlier sets emptied (indices preserved); restored immediately after.
    import concourse.bacc as bacc_mod

    orig_tables = bacc_mod.get_activation_tables
    def _tables_set6(arch):
        t = orig_tables(arch)
        return {
            k: (v if i == 6 else set()) for i, (k, v) in enumerate(t.items())
        }

    bacc_mod.get_activation_tables = _tables_set6
    try:
        nc.finalize()
    finally:
        bacc_mod.get_activation_tables = orig_tables
    return nc


def get_nc(with_bias: bool = False, repeat: int = 1, **kw):
    key = ("nc", with_bias, repeat, tuple(sorted(kw.items())))
    if key not in _CACHE:
        _CACHE[key] = _build(with_bias, repeat=repeat, **kw)
    return _CACHE[key]


def make_in_maps(x: np.ndarray, W: np.ndarray, b: np.ndarray, with_bias: bool = False):
    import ml_dtypes

    f8 = ml_dtypes.float8_e4m3
    xs = x.astype(np.float16)
    xts = np.ascontiguousarray(xs.T)
    in_maps = []
    for i in range(N_CORES):
        lo, hi = i * B_LOC, (i + 1) * B_LOC
        m = {"xt_h": np.ascontiguousarray(xts[:, lo:hi])}
        if X8:
            m["x8_h"] = np.ascontiguousarray(xs[lo:hi]).astype(f8)
        else:
            m["x_h"] = np.ascontiguousarray(xs[lo:hi])
        if FP8:
            m["xt8_h"] = np.ascontiguousarray(xts[:, lo:hi]).astype(f8)
            m["w8_h"] = (W * W_SCALE).astype(f8)
        else:
            m["w_h"] = W.astype(np.float16)
        if with_bias:
            m["b_h"] = (b * (W_SCALE if FP8 else 1.0)).astype(np.float16).reshape(1, C)
        in_maps.append(m)
    return in_maps


def kernel(x: np.ndarray, W: np.ndarray, b: np.ndarray) -> np.ndarray:
    from concourse.bass_utils import run_bass_kernel_spmd

    x, W, b = np.asarray(x), np.asarray(W), np.asarray(b)
    with_bias = bool(np.any(b))
    nc = get_nc(with_bias)
    in_maps = make_in_maps(x, W, b, with_bias)
    res = run_bass_kernel_spmd(nc, in_maps, list(range(N_CORES))).results
    out = np.concatenate(
        [
            np.asarray(res[i]["out_f"], dtype=np.float32).reshape(-1)
            for i in range(N_CORES)
        ]
    )
    return out



# revision 50
# speedup vs baseline: 1.0919x; 1.0919x over previous
"""Trainium2 Bass kernel for nn_CROSSLoss (softmax-entropy * mean-cosine-similarity loss).

Math (reference):
    logits = x @ W + b                       [B, C]
    loss_i = sum_n softmax(logits)_in * log_softmax(logits)_in
    xn     = x / max(||x_i||, eps)
    weight_i = (1/B) * sum_j xn_i . xn_j
    out_i  = loss_i * weight_i

Key restructurings:
  * weight_i = xn_i . s / B with s = sum_j xn_j -- the BxB similarity matrix
    is never materialized; the per-core partial of the [D] vector s is the
    only cross-core data. It moves via ONE AllGather (4KB/core) + local tree
    reduce: one ring pass instead of an AllReduce's two.
  * logits are small (|l| < 4), so softmax needs no max subtraction:
    loss = S2/Z - ln Z with Z = sum e^l and S2 = sum l*e^l.
  * S2 options: "eps" computes S2/Z by central difference from two ACT
    exp-accum passes (za = sum e^{(1+eps)l}, zb = sum e^{(1-eps)l};
    S2/Z = (za-zb)/(eps(za+zb)), O(eps^2) bias ~1e-5); "dve" uses the
    direct exp + multiply + reduce.
  * The logits GEMM runs in fp8 (e4m3) with MatmulPerfMode.DoubleRow: two
    128-deep contraction tiles per pass, 2x PE throughput. W is pre-scaled
    by 256 on the host into e4m3's normal range; the 1/256 rides in the
    exp scale and the final loss arithmetic.
  * Pipelining across repeat iterations: the big input tiles and the small
    per-row stat tiles rotate through bufs=2 pools, so iteration i+1's DMAs
    and logits fill iteration i's collective bubble. Bulk inputs ride the
    SP HWDGE queue; collective-adjacent small DMAs ride the gpsimd SWDGE
    queue (engine DMA FIFOs are in-order -- a collective-gated DMA on the
    input queue would stall the next iteration's whole input burst).
  * Phase A is a per-chunk pipeline square -> r (ACT ln/exp) -> rank-1 s
    matmul, so the collective input is ready right after the last x chunk
    lands. Stats fuse the product+reduce into one DVE scalar_tensor_tensor
    pass with accum_out.
  * HAM keep-alive: tiny matmuls dep-chained on late stats/loss results
    tick the PE through the tail window so the idle-window detector does
    not re-throttle the clock to K=4/8.
  * Tail stays in row space: t1 = loss*r/B transposes through DRAM during
    the collective; the final multiply reads u straight from PSUM.

Sharding: data-parallel over batch; 1024 rows/core. Per core loads: x rows
f16 (norm/s path), xt f16 (u matmul), xt fp8 + W fp8 (logits), 6MB total.
"""

import numpy as np

N_CORES = 8
B, D, C = 8192, 1024, 1000
B_LOC = B // N_CORES  # rows per core
P = 128
RC = B_LOC // P  # row chunks per core
KC = D // P  # contraction chunks
N0 = 512
N1 = C - N0
W_SCALE = 256.0  # host-side W pre-scale for fp8 range
R_SCALE = 64.0  # host-independent r pre-scale for fp8 s matmul
S2_EPS = 0.01  # central-difference step for S2
# linear seed y0 = C1/ss + C0 for rsqrt over ss in [810, 1240] (2.8e-3 max
# rel; one Newton iteration brings it to 1.2e-5)
RSQRT_C1 = 15.78414098
RSQRT_C0 = 0.01574952754

_CACHE = {}

# knobs for A/B experiments
COLLECTIVE = "allreduce"  # "allreduce" | "allgather" (HW: same ~2.3us, AR has shorter tail)
FP8 = True  # fp8 DoubleRow logits matmul
X8 = False  # phase A on fp8 x: rejected, sqrt(B)-amplified s noise (2.6% weight err)
S2_MODE = "dve"  # "dve" (exp + mul + reduce) | "eps" (2 exp passes)
DVE_SS = (1, 3, 5)  # phase-A chunks squared on DVE instead of ACT
NEWTON_ITERS = 1
R_MODE = "lnexp"  # "lnexp" (ACT exp(-.5 ln ss)) | "newton"
PE_FILLER = 0  # dummy rank-1 matmul pairs keeping PE warm pre-u (HW: hurts)
LOGITS_HINT_US = 0.0  # scheduler hint: delay logits c1.. past s matmuls (HW: off)


def _build(
    with_bias: bool,
    repeat: int = 1,
    collective: str = COLLECTIVE,
    fp8: bool = FP8,
    x8: bool = X8,
    s2_mode: str = S2_MODE,
    dve_ss: tuple = DVE_SS,
    newton_iters: int = NEWTON_ITERS,
    r_mode: str = R_MODE,
    pe_filler: int = PE_FILLER,
    logits_hint_us: float = LOGITS_HINT_US,
    debug: bool = False,
    hw_loop: int = 0,
    chain: bool = False,
    timing_build: bool = False,
    skip: tuple = (),
):
    from contextlib import ExitStack

    import concourse.bacc as bacc
    import concourse.tile as tile
    from concourse import mybir

    f8 = mybir.dt.float8e4
    bf16 = mybir.dt.bfloat16
    f16 = mybir.dt.float16
    f32 = mybir.dt.float32
    Alu = mybir.AluOpType
    Act = mybir.ActivationFunctionType
    DR = mybir.MatmulPerfMode.DoubleRow

    nc = bacc.Bacc(None, num_devices=N_CORES)

    x_dt = f8 if x8 else f16
    if timing_build:
        # timing-only: big inputs live in internal DRAM (uninitialized), so
        # per-call dispatch ships no data and wall-clock noise stays tiny
        x_h = nc.dram_tensor("x_t", [B_LOC, D], x_dt)
        xt_h = nc.dram_tensor("xt_t", [D, B_LOC], f16)
        if fp8:
            xt8_h = nc.dram_tensor("xt8_t", [D, B_LOC], f8)
            w_h = nc.dram_tensor("w8_t", [D, C], f8)
        else:
            w_h = nc.dram_tensor("w_t", [D, C], f16)
    else:
        x_h = nc.declare_dram_parameter(
            "x8_h" if x8 else "x_h", [B_LOC, D], x_dt, isOutput=False
        )
        xt_h = nc.declare_dram_parameter("xt_h", [D, B_LOC], f16, isOutput=False)
        if fp8:
            xt8_h = nc.declare_dram_parameter("xt8_h", [D, B_LOC], f8, isOutput=False)
            w_h = nc.declare_dram_parameter("w8_h", [D, C], f8, isOutput=False)
        else:
            w_h = nc.declare_dram_parameter("w_h", [D, C], f16, isOutput=False)
    b_h = (
        nc.declare_dram_parameter("b_h", [1, C], f16, isOutput=False)
        if with_bias
        else None
    )
    out_f = nc.declare_dram_parameter("out_f", [1, B_LOC], f32, isOutput=True)
    if debug:
        dbg_za = nc.declare_dram_parameter("dbg_za", [P, RC], f32, isOutput=True)
        dbg_zb = nc.declare_dram_parameter("dbg_zb", [P, RC], f32, isOutput=True)
        dbg_r = nc.declare_dram_parameter("dbg_r", [P, RC], f32, isOutput=True)
        dbg_s = nc.declare_dram_parameter("dbg_s", [P, KC], f32, isOutput=True)
        dbg_u = nc.declare_dram_parameter("dbg_u", [1, B_LOC], f32, isOutput=True)
        dbg_u2 = nc.declare_dram_parameter("dbg_u2", [1, B_LOC], f32, isOutput=True)
        dbg_u3 = nc.declare_dram_parameter("dbg_u3", [1, C], f32, isOutput=True)
        dbg_t1 = nc.declare_dram_parameter("dbg_t1", [1, B_LOC], f32, isOutput=True)

    cc_dt = f16
    # double-buffered across repeat iterations so iteration i+1's collective
    # input write never waits on iteration i's collective completion
    cc_ins = [nc.dram_tensor(f"cc_in{j}", [1, D], cc_dt) for j in range(2)]
    if collective == "allgather":
        cc_outs = [
            nc.dram_tensor(f"cc_out{j}", [N_CORES, D], cc_dt, addr_space="Shared")
            for j in range(2)
        ]
    else:
        cc_outs = [
            nc.dram_tensor(f"cc_out{j}", [1, D], cc_dt, addr_space="Shared")
            for j in range(2)
        ]
    t1_drams = [nc.dram_tensor(f"t1_dram{j}", [1, B_LOC], f32) for j in range(2)]

    w_dt = f8 if fp8 else f16
    l_scale = (1.0 / W_SCALE) if fp8 else 1.0

    with tile.TileContext(nc) as tc, ExitStack() as ctx:
        singles = ctx.enter_context(tc.tile_pool(name="singles", bufs=1))
        # big per-iteration inputs double-buffer so iteration i+1's DMAs and
        # logits overlap iteration i's collective bubble
        bigs = ctx.enter_context(tc.tile_pool(name="bigs", bufs=2))
        work = ctx.enter_context(tc.tile_pool(name="work", bufs=3))
        lps = ctx.enter_context(tc.tile_pool(name="lps", bufs=2, space="PSUM"))
        vps = ctx.enter_context(tc.tile_pool(name="vps", bufs=1, space="PSUM"))

        if with_bias:
            b_sb = singles.tile([1, C], f16)
            ones = singles.tile([1, P], f16)

        # newton-path + eps-path temporaries (unused by default)
        rs_h = singles.tile([P, RC], f32)
        rs_a = singles.tile([P, RC], f32)
        rs_b = singles.tile([P, RC], f32)
        zs_all = singles.tile([P, RC], f32)
        zd_all = singles.tile([P, RC], f32)
        # small per-iteration stat tiles rotate (bufs=2) so iteration i+1's
        # phase A never waits on iteration i's loss-math readers
        stats = ctx.enter_context(tc.tile_pool(name="stats", bufs=2))

        def stat_tiles():
            ss_all = stats.tile([P, RC], f32, tag="ss", name="ss_all")
            rs_g = stats.tile([P, RC], f32, tag="rsg", name="rs_g")
            r_all = stats.tile([P, RC], f32, tag="r", name="r_all")
            r16 = stats.tile([P, RC], f16, tag="r16", name="r16")
            za_all = stats.tile([P, RC], f32, tag="za", name="za_all")
            zb_all = stats.tile([P, RC], f32, tag="zb", name="zb_all")
            lnz = stats.tile([P, RC], f32, tag="lnz", name="lnz")
            rz = stats.tile([P, RC], f32, tag="rz", name="rz")
            s2z = stats.tile([P, RC], f32, tag="s2z", name="s2z")
            loss = stats.tile([P, RC], f32, tag="loss", name="loss")
            t1 = stats.tile([P, RC], f32, tag="t1", name="t1")
            t1_row = stats.tile([1, B_LOC], f32, tag="t1r", name="t1_row")
            out_row = stats.tile([1, B_LOC], f32, tag="or", name="out_row")
            s16 = stats.tile([P, KC], f16, tag="s16", name="s16")
            s_row = stats.tile([1, D], f16, tag="srow", name="s_row")
            return (ss_all, rs_g, r_all, r16, za_all, zb_all, lnz, rz, s2z,
                    loss, t1, t1_row, out_row, s16, s_row)
        if collective == "allgather":
            s_g = singles.tile([P, N_CORES, KC], f16)  # [p][src core][k]
            s_t4 = singles.tile([P, 4, KC], f32)
            s_t2 = singles.tile([P, 2, KC], f32)
            s_gr = singles.tile([P, 1, KC], f32)
        if collective != "allgather":
            s_f32 = singles.tile([P, KC], f32)

        s_ps = vps.tile([1, D], f32, tag="s", name="s_ps")
        u_ps = vps.tile([1, B_LOC], f32, tag="u", name="u_ps")
        if chain:
            chain_sb = singles.tile([1, 8], f32)

        if with_bias:
            nc.vector.memset(ones, 1.0)

        def logits_chunk(c, xt8_sb, xt_sb, w_sb):
            lpsum = lps.tile([P, C], f32, tag="logits")
            last_k_stops = not with_bias
            if fp8:
                for kk in range(KC // 2):
                    lt = xt8_sb[:, 2 * kk : 2 * kk + 2, c * P : (c + 1) * P]
                    nc.tensor.matmul(
                        lpsum[:, 0:N0],
                        lhsT=lt,
                        rhs=w_sb[:, 2 * kk : 2 * kk + 2, 0:N0],
                        start=(kk == 0),
                        stop=(last_k_stops and kk == KC // 2 - 1),
                        perf_mode=DR,
                    )
                    nc.tensor.matmul(
                        lpsum[:, N0:C],
                        lhsT=lt,
                        rhs=w_sb[:, 2 * kk : 2 * kk + 2, N0:C],
                        start=(kk == 0),
                        stop=(last_k_stops and kk == KC // 2 - 1),
                        perf_mode=DR,
                    )
            else:
                for k in range(KC):
                    lt = xt_sb[:, k, c * P : (c + 1) * P]
                    nc.tensor.matmul(
                        lpsum[:, 0:N0],
                        lhsT=lt,
                        rhs=w_sb[:, k, 0:N0],
                        start=(k == 0),
                        stop=(last_k_stops and k == KC - 1),
                    )
                    nc.tensor.matmul(
                        lpsum[:, N0:C],
                        lhsT=lt,
                        rhs=w_sb[:, k, N0:C],
                        start=(k == 0),
                        stop=(last_k_stops and k == KC - 1),
                    )
            if with_bias:
                nc.tensor.matmul(
                    lpsum[:, 0:N0],
                    lhsT=ones,
                    rhs=b_sb[:, 0:N0],
                    start=False,
                    stop=True,
                    skip_group_check=True,
                )
                nc.tensor.matmul(
                    lpsum[:, N0:C],
                    lhsT=ones,
                    rhs=b_sb[:, N0:C],
                    start=False,
                    stop=True,
                    skip_group_check=True,
                )
            return lpsum

        def stats_chunk(c, lpsum):
            if s2_mode == "eps":
                ea = work.tile([P, C], bf16, tag="ea")
                nc.scalar.activation(
                    out=ea,
                    in_=lpsum,
                    func=Act.Exp,
                    scale=l_scale * (1.0 + S2_EPS),
                    accum_out=za_all[:, c : c + 1],
                )
                nc.scalar.activation(
                    out=ea,
                    in_=lpsum,
                    func=Act.Exp,
                    scale=l_scale * (1.0 - S2_EPS),
                    accum_out=zb_all[:, c : c + 1],
                )
            else:
                e_t = work.tile([P, C], bf16, tag="e")
                nc.scalar.activation(
                    out=e_t,
                    in_=lpsum,
                    func=Act.Exp,
                    scale=l_scale,
                    accum_out=za_all[:, c : c + 1],
                )
                prod = work.tile([P, C], bf16, tag="prod")
                nc.vector.scalar_tensor_tensor(
                    out=prod,
                    in0=lpsum,
                    scalar=1.0,
                    in1=e_t,
                    op0=Alu.mult,
                    op1=Alu.mult,
                    accum_out=zb_all[:, c : c + 1],
                )

        from contextlib import nullcontext

        loop_cm = tc.For_i(0, hw_loop) if hw_loop else nullcontext(0)
        pending_tail = None
        with loop_cm:
          for it in range(repeat):
              cc_in = cc_ins[it % 2]
              cc_out = cc_outs[it % 2]
              t1_dram = t1_drams[it % 2]
              (ss_all, rs_g, r_all, r16, za_all, zb_all, lnz, rz, s2z,
               loss, t1, t1_row, out_row, s16, s_row) = stat_tiles()
              # rotating per-iteration input tiles
              x_sb = bigs.tile([P, RC, D], x_dt, tag="x", name="x_sb")
              xt_sb = bigs.tile([P, KC, B_LOC], f16, tag="xt", name="xt_sb")
              xt8_sb = (
                  bigs.tile([P, KC, B_LOC], f8, tag="xt8", name="xt8_sb")
                  if fp8
                  else None
              )
              w_sb = bigs.tile([P, KC, C], w_dt, tag="w", name="w_sb")
              # ---- input DMAs ----
              # ALL bulk inputs ride the SP HWDGE queue: x first (it gates the
              # latency-critical chain ss -> r -> s -> collective), then the
              # fp8 logits operands, then xt16 last (only needed post-collective
              # for the u matmul). The gpsimd SWDGE queue is reserved for the
              # collective-adjacent small DMAs so the input stream never queues
              # behind a collective-gated wait (engine DMA FIFOs are in-order).
              need_x = (not ({"a", "s"} <= set(skip))) or pe_filler > 0
              for c in (range(RC) if need_x else ()):
                  nc.sync.dma_start(out=x_sb[:, c, :], in_=x_h[c * P : (c + 1) * P, :])
              # NB: a [:, k:k+2, :] SBUF destination iterates [p][j][i], so the
              # DRAM side must supply rows in (p j) order -- a plain [256, D]
              # slice would interleave row pairs.
              for k in ([] if "c" in skip else range(0, KC, 2)):
                  nc.sync.dma_start(
                      out=w_sb[:, k : k + 2, :],
                      in_=w_h[k * P : (k + 2) * P, :].rearrange("(j p) i -> p j i", j=2),
                  )
                  if fp8:
                      nc.sync.dma_start(
                          out=xt8_sb[:, k : k + 2, :],
                          in_=xt8_h[k * P : (k + 2) * P, :].rearrange(
                              "(j p) i -> p j i", j=2
                          ),
                      )
              for k in ([] if "u" in skip else range(0, KC, 2)):
                  nc.sync.dma_start(
                      out=xt_sb[:, k : k + 2, :],
                      in_=xt_h[k * P : (k + 2) * P, :].rearrange("(j p) i -> p j i", j=2),
                  )
              if with_bias:
                  nc.sync.dma_start(out=b_sb, in_=b_h[:, :])

              # ---- Phase A: per-chunk pipeline square -> r -> s matmul so
              # the collective input is ready as soon as the last x chunk
              # lands (r per chunk instead of one [P, RC] batch)
              if "a" in skip:
                  nc.vector.memset(r16, 0.01)
                  nc.vector.memset(r_all, 0.01)
              for c in ([] if "a" in skip else range(RC)):
                  sq = work.tile([P, D], f16, tag="sq")
                  if c not in dve_ss:
                      nc.scalar.activation(
                          out=sq,
                          in_=x_sb[:, c, :],
                          func=Act.Square,
                          accum_out=ss_all[:, c : c + 1],
                      )
                  else:
                      nc.vector.scalar_tensor_tensor(
                          out=sq,
                          in0=x_sb[:, c, :],
                          scalar=1.0,
                          in1=x_sb[:, c, :],
                          op0=Alu.mult,
                          op1=Alu.mult,
                          accum_out=ss_all[:, c : c + 1],
                      )
                  if r_mode == "lnexp":
                      nc.scalar.activation(
                          out=rs_g[:, c : c + 1],
                          in_=ss_all[:, c : c + 1],
                          func=Act.Ln,
                      )
                      nc.scalar.activation(
                          out=r_all[:, c : c + 1],
                          in_=rs_g[:, c : c + 1],
                          func=Act.Exp,
                          scale=-0.5,
                      )
                      nc.vector.tensor_copy(
                          out=r16[:, c : c + 1], in_=r_all[:, c : c + 1]
                      )
                  if "s" not in skip:
                      nc.tensor.matmul(
                          s_ps[:, 0:512],
                          lhsT=r16[:, c : c + 1],
                          rhs=x_sb[:, c, 0:512],
                          start=(c == 0),
                          stop=(c == RC - 1),
                      )
                      nc.tensor.matmul(
                          s_ps[:, 512:1024],
                          lhsT=r16[:, c : c + 1],
                          rhs=x_sb[:, c, 512:1024],
                          start=(c == 0),
                          stop=(c == RC - 1),
                      )
              # newton fallback (unused by default)
              if "a" not in skip and r_mode != "lnexp":
               nc.vector.tensor_scalar_mul(out=rs_h, in0=ss_all, scalar1=0.5)
               nc.vector.reciprocal(out=rs_g, in_=ss_all)
               nc.vector.tensor_scalar(
                   out=r_all,
                   in0=rs_g,
                   scalar1=RSQRT_C1,
                   scalar2=RSQRT_C0,
                   op0=Alu.mult,
                   op1=Alu.add,
               )
               for _i in (range(newton_iters) if r_mode != "lnexp" else []):
                   nc.vector.tensor_tensor(out=rs_a, in0=r_all, in1=r_all, op=Alu.mult)
                   nc.vector.tensor_tensor(out=rs_b, in0=rs_h, in1=rs_a, op=Alu.mult)
                   nc.vector.tensor_scalar(
                       out=rs_b,
                       in0=rs_b,
                       scalar1=-1.0,
                       scalar2=1.5,
                       op0=Alu.mult,
                       op1=Alu.add,
                   )
                   nc.vector.tensor_tensor(out=r_all, in0=r_all, in1=rs_b, op=Alu.mult)
               if r_mode != "lnexp":
                   nc.vector.tensor_copy(out=r16, in_=r_all)

              # ---- Phase B: share s across the 8 cores ----
              # s_row holds s in d=(k p) order; cc_in is written (p k)-transposed
              # so the post-collective gather reads 32B-contiguous runs.
              if "s" in skip:
                  nc.vector.memset(s16, 0.01)
              elif repeat == 1:
                  # halves cast/written separately: the half-A cast and cc
                  # write overlap the half-B s matmuls on the critical
                  # collective-issue chain
                  nc.vector.tensor_copy(
                      out=s_row[0:1, 0:512], in_=s_ps[0:1, 0:512]
                  )
                  nc.vector.tensor_copy(
                      out=s_row[0:1, 512:1024], in_=s_ps[0:1, 512:1024]
                  )
              elif True:
                  nc.vector.tensor_copy(out=s_row, in_=s_ps)
              if "s" in skip:
                  pass
              elif collective == "allgather":
                  nc.gpsimd.dma_start(
                      out=cc_in[0, :].rearrange("(p k) -> k p", k=KC),
                      in_=s_row[:, :],
                  )
                  nc.gpsimd.collective_compute(
                      "AllGather",
                      mybir.AluOpType.bypass,
                      replica_groups=[list(range(N_CORES))],
                      ins=[cc_in[:, :]],
                      outs=[cc_out[:, :]],
                  )
                  nc.gpsimd.dma_start(
                      out=s_g[:, :, :],
                      in_=cc_out[:, :].rearrange("c (p k) -> p c k", k=KC),
                  )
                  nc.vector.tensor_tensor(
                      out=s_t4, in0=s_g[:, 0:4, :], in1=s_g[:, 4:8, :], op=Alu.add
                  )
                  nc.vector.tensor_tensor(
                      out=s_t2, in0=s_t4[:, 0:2, :], in1=s_t4[:, 2:4, :], op=Alu.add
                  )
                  nc.vector.tensor_tensor(
                      out=s_gr, in0=s_t2[:, 0:1, :], in1=s_t2[:, 1:2, :], op=Alu.add
                  )
                  nc.vector.tensor_copy(out=s16, in_=s_gr[:, 0, :])
              elif collective == "none":
                  # timing probe only: s = own partial (wrong result)
                  nc.gpsimd.dma_start(out=cc_in[:, :], in_=s_row[:, :])
              else:
                  if repeat == 1:
                      nc.gpsimd.dma_start(
                          out=cc_in[0:1, 0:512], in_=s_row[0:1, 0:512]
                      )
                      nc.gpsimd.dma_start(
                          out=cc_in[0:1, 512:1024], in_=s_row[0:1, 512:1024]
                      )
                  else:
                      nc.gpsimd.dma_start(out=cc_in[:, :], in_=s_row[:, :])
                  nc.gpsimd.collective_compute(
                      "AllReduce",
                      mybir.AluOpType.add,
                      replica_groups=[list(range(N_CORES))],
                      ins=[cc_in[:, :]],
                      outs=[cc_out[:, :]],
                  )

              # ---- Phase C: remaining logits + stats ----
              if "c" in skip:
                  nc.vector.memset(za_all, 1000.0)
                  nc.vector.memset(zb_all, 999.0)
              else:
                  for c in range(RC):
                      lpsum = logits_chunk(c, xt8_sb, xt_sb, w_sb)
                      stats_chunk(c, lpsum)

              # loss math (column layout [P, RC])
              if "l" in skip:
                  nc.vector.memset(t1_row, 0.5)
                  nc.vector.memset(out_row, 0.5)
              elif s2_mode == "eps":
                  # Z = (za+zb)/2, S2/Z = (za-zb)/(eps*(za+zb))
                  nc.vector.tensor_tensor(out=zs_all, in0=za_all, in1=zb_all, op=Alu.add)
                  nc.vector.tensor_tensor(
                      out=zd_all, in0=za_all, in1=zb_all, op=Alu.subtract
                  )
                  nc.scalar.activation(out=lnz, in_=zs_all, func=Act.Ln, scale=0.5)
                  nc.vector.reciprocal(out=rz, in_=zs_all)
                  nc.vector.tensor_tensor(out=s2z, in0=zd_all, in1=rz, op=Alu.mult)
                  nc.vector.scalar_tensor_tensor(
                      out=loss,
                      in0=s2z,
                      scalar=1.0 / S2_EPS,
                      in1=lnz,
                      op0=Alu.mult,
                      op1=Alu.subtract,
                  )
              elif True:
                  # za = Z, zb = S2' = W_SCALE * S2.
                  # ln Z on DVE as ln(1280) + ln1p(y), y = Z/1280 - 1 (|y|<0.15
                  # for these inputs; quartic error ~1e-5). Avoids the ACT Ln
                  # table swap and keeps the whole tail on one engine.
                  nc.scalar.activation(out=lnz, in_=za_all, func=Act.Ln)
                  nc.vector.reciprocal(out=rz, in_=za_all)
                  nc.vector.tensor_tensor(out=s2z, in0=zb_all, in1=rz, op=Alu.mult)
                  nc.vector.scalar_tensor_tensor(
                      out=loss,
                      in0=s2z,
                      scalar=l_scale,
                      in1=lnz,
                      op0=Alu.mult,
                      op1=Alu.subtract,
                  )
              if "l" not in skip:
                  nc.vector.scalar_tensor_tensor(
                      out=t1,
                      in0=loss,
                      scalar=1.0 / B,
                      in1=r_all,
                      op0=Alu.mult,
                      op1=Alu.mult,
                  )
                  # t1 -> row layout through DRAM while the collective flies
                  t_eng = nc.scalar if repeat == 1 else nc.gpsimd
                  t_eng.dma_start(
                      out=t1_dram[0, :].rearrange("(c p) -> p c", p=P), in_=t1[:, :]
                  )
                  t_eng.dma_start(out=t1_row[:, :], in_=t1_dram[:, :])

              # PE keep-warm filler: rank-1 matmuls on resident data, overwritten
              # by the real u accumulation group (start=True resets the banks).
              for i in range(pe_filler):
                  cc = i % RC
                  nc.tensor.matmul(
                      u_ps[:, 0:512],
                      lhsT=r16[:, cc : cc + 1],
                      rhs=x_sb[:, cc, 0:512],
                      start=(i == 0),
                      stop=(i == pe_filler - 1),
                  )
                  nc.tensor.matmul(
                      u_ps[:, 512:1024],
                      lhsT=r16[:, cc : cc + 1],
                      rhs=x_sb[:, cc, 512:1024],
                      start=(i == 0),
                      stop=(i == pe_filler - 1),
                  )

              # HAM keep-alive: tiny matmuls dep-chained on late stats/loss
              # results fire ~1-2us apart through the tail window, preventing
              # the PE idle-window detector from re-throttling to K=4/8.
              if "c" not in skip and "l" not in skip:
                  for fc in (5, 6, 7):
                      nc.tensor.matmul(
                          u_ps[0:1, 0:RC],
                          lhsT=zb_all[:, fc : fc + 1],
                          rhs=zb_all[:, 0:RC],
                          start=True,
                          stop=True,
                          skip_group_check=True,
                      )
                  nc.tensor.matmul(
                      u_ps[0:1, 0:RC],
                      lhsT=loss[:, 0:1],
                      rhs=loss[:, 0:RC],
                      start=True,
                      stop=True,
                      skip_group_check=True,
                  )
                  nc.tensor.matmul(
                      u_ps[0:1, 0:RC],
                      lhsT=t1[:, 0:1],
                      rhs=t1[:, 0:RC],
                      start=True,
                      stop=True,
                      skip_group_check=True,
                  )
                  nc.tensor.matmul(
                      u_ps[0:1, 0:RC],
                      lhsT=t1_row[0:1, 0:1],
                      rhs=t1_row[0:1, 0:RC],
                      start=True,
                      stop=True,
                      skip_group_check=True,
                  )

              # ---- Phase D (deferred tail): gather s, u = x @ s, out ----
              def make_tail(s16, xt_sb, t1_row, out_row, cc_out, cc_in):
                  def tail():
                      if "s" not in skip and collective in ("allreduce", "none"):
                          src_cc = cc_out if collective == "allreduce" else cc_in
                          # single-iteration build (the deployed kernel): the
                          # ACT queue has no later compute to block, so use the
                          # lower-latency HWDGE path for the gather
                          g_eng = nc.scalar if repeat == 1 else nc.gpsimd
                          g_eng.dma_start(
                              out=s16[:, 0:KC],
                              in_=src_cc[0, 0:D].rearrange("(k p) -> p k", p=P),
                          )
                      if "u" in skip:
                          nc.vector.memset(u_ps, 1.0)
                      o_eng = nc.scalar if repeat == 1 else nc.gpsimd
                      if repeat == 1:
                          # single-shot: half A fully accumulates first so its
                          # multiply and output write overlap half B's matmuls
                          for lo, hi in ((0, 512), (512, 1024)):
                              for k in ([] if "u" in skip else range(KC)):
                                  nc.tensor.matmul(
                                      u_ps[:, lo:hi],
                                      lhsT=s16[:, k : k + 1],
                                      rhs=xt_sb[:, k, lo:hi],
                                      start=(k == 0),
                                      stop=(k == KC - 1),
                                  )
                              if "l" not in skip:
                                  nc.vector.tensor_tensor(
                                      out=out_row[0:1, lo:hi],
                                      in0=u_ps[0:1, lo:hi],
                                      in1=t1_row[0:1, lo:hi],
                                      op=Alu.mult,
                                  )
                              o_eng.dma_start(
                                  out=out_f[0:1, lo:hi], in_=out_row[0:1, lo:hi]
                              )
                      else:
                          for k in ([] if "u" in skip else range(KC)):
                              nc.tensor.matmul(
                                  u_ps[:, 0:512],
                                  lhsT=s16[:, k : k + 1],
                                  rhs=xt_sb[:, k, 0:512],
                                  start=(k == 0),
                                  stop=(k == KC - 1),
                              )
                              nc.tensor.matmul(
                                  u_ps[:, 512:1024],
                                  lhsT=s16[:, k : k + 1],
                                  rhs=xt_sb[:, k, 512:1024],
                                  start=(k == 0),
                                  stop=(k == KC - 1),
                              )
                          if "l" not in skip:
                              nc.vector.tensor_tensor(
                                  out=out_row, in0=u_ps, in1=t1_row, op=Alu.mult
                              )
                          o_eng.dma_start(out=out_f[:, :], in_=out_row[:, :])
                  return tail

              this_tail = make_tail(s16, xt_sb, t1_row, out_row, cc_out, cc_in)
              this_tail()
              if chain:
                  # serialize repeat iterations: read the output back and
                  # scribble on x_sb so the next iteration's x DMA must wait
                  nc.sync.dma_start(out=chain_sb, in_=out_f[0:1, 0:8])
                  nc.vector.tensor_copy(out=x_sb[0:1, 0, 0:8], in_=chain_sb)

              if debug:
                  u_dbg_row = singles.tile([1, B_LOC], f32)
                  s_dbg = singles.tile([P, KC], f32)
                  nc.vector.tensor_copy(out=u_dbg_row, in_=u_ps)
                  nc.vector.tensor_copy(out=s_dbg, in_=s16)
                  # v2: weights from standalone [128,1] tiles
                  s16k = [singles.tile([P, 1], f16, name=f"s16k{k}") for k in range(KC)]
                  for k in range(KC):
                      nc.vector.tensor_copy(out=s16k[k], in_=s16[:, k : k + 1])
                  for k in range(KC):
                      nc.tensor.matmul(
                          s_ps[:, 0:512], lhsT=s16k[k], rhs=xt_sb[:, k, 0:512],
                          start=(k == 0), stop=(k == KC - 1),
                      )
                      nc.tensor.matmul(
                          s_ps[:, 512:1024], lhsT=s16k[k], rhs=xt_sb[:, k, 512:1024],
                          start=(k == 0), stop=(k == KC - 1),
                      )
                  u2_row = singles.tile([1, B_LOC], f32)
                  nc.vector.tensor_copy(out=u2_row, in_=s_ps)
                  nc.sync.dma_start(out=dbg_u2[:, :], in_=u2_row[:, :])
                  # v3: bf16 weights from a [128, KC] tile
                  s16b = singles.tile([P, KC], bf16)
                  xtb = singles.tile([P, 1024], bf16)
                  nc.vector.tensor_copy(out=s16b, in_=s16)
                  u3_ps = lps.tile([P, C], f32, tag="logits")
                  for k in range(KC):
                      nc.tensor.matmul(
                          u3_ps[0:1, 0:512], lhsT=s16b[:, k : k + 1], rhs=xt_sb[:, k, 0:512],
                          start=(k == 0), stop=(k == KC - 1), skip_group_check=True,
                      )
                      nc.tensor.matmul(
                          u3_ps[0:1, 512:1000], lhsT=s16b[:, k : k + 1], rhs=xt_sb[:, k, 512:1000],
                          start=(k == 0), stop=(k == KC - 1), skip_group_check=True,
                      )
                  u3_row = singles.tile([1, C], f32)
                  nc.vector.tensor_copy(out=u3_row, in_=u3_ps[0:1, :])
                  nc.sync.dma_start(out=dbg_u3[:, :], in_=u3_row[:, :])
                  nc.sync.dma_start(out=dbg_za[:, :], in_=za_all)
                  nc.sync.dma_start(out=dbg_zb[:, :], in_=zb_all)
                  nc.sync.dma_start(out=dbg_r[:, :], in_=r_all)
                  nc.sync.dma_start(out=dbg_s[:, :], in_=s_dbg)
                  nc.sync.dma_start(out=dbg_u[:, :], in_=u_dbg_row)
                  nc.sync.dma_start(out=dbg_t1[:, :], in_=t1_row)

    # Steer every activation onto act-func-set 6 (natural_log_exp_and_others:
    # Exp+Ln+Square+Copy in ONE table) so the kernel runs with a single
    # LoadActFuncSet and zero mid-stream table swaps. The selection pass takes
    # the first set containing each func, so present it a view with the
    # earlier sets emptied (indices preserved); restored immediately after.
    import concourse.bacc as bacc_mod

    orig_tables = bacc_mod.get_activation_tables
    def _tables_set6(arch):
        t = orig_tables(arch)
        return {
            k: (v if i == 6 else set()) for i, (k, v) in enumerate(t.items())
        }

    bacc_mod.get_activation_tables = _tables_set6
    try:
        nc.finalize()
    finally:
        bacc_mod.get_activation_tables = orig_tables
    return nc


def get_nc(with_bias: bool = False, repeat: int = 1, **kw):
    key = ("nc", with_bias, repeat, tuple(sorted(kw.items())))
    if key not in _CACHE:
        _CACHE[key] = _build(with_bias, repeat=repeat, **kw)
    return _CACHE[key]


def make_in_maps(x: np.ndarray, W: np.ndarray, b: np.ndarray, with_bias: bool = False):
    import ml_dtypes

    f8 = ml_dtypes.float8_e4m3
    xs = x.astype(np.float16)
    xts = np.ascontiguousarray(xs.T)
    in_maps = []
    for i in range(N_CORES):
        lo, hi = i * B_LOC, (i + 1) * B_LOC
        m = {"xt_h": np.ascontiguousarray(xts[:, lo:hi])}
        if X8:
            m["x8_h"] = np.ascontiguousarray(xs[lo:hi]).astype(f8)
        else:
            m["x_h"] = np.ascontiguousarray(xs[lo:hi])
        if FP8:
            m["xt8_h"] = np.ascontiguousarray(xts[:, lo:hi]).astype(f8)
            m["w8_h"] = (W * W_SCALE).astype(f8)
        else:
            m["w_h"] = W.astype(np.float16)
        if with_bias:
            m["b_h"] = (b * (W_SCALE if FP8 else 1.0)).astype(np.float16).reshape(1, C)
        in_maps.append(m)
    return in_maps


def kernel(x: np.ndarray, W: np.ndarray, b: np.ndarray) -> np.ndarray:
    from concourse.bass_utils import run_bass_kernel_spmd

    x, W, b = np.asarray(x), np.asarray(W), np.asarray(b)
    with_bias = bool(np.any(b))
    nc = get_nc(with_bias)
    in_maps = make_in_maps(x, W, b, with_bias)
    res = run_bass_kernel_spmd(nc, in_maps, list(range(N_CORES))).results
    out = np.concatenate(
        [
            np.asarray(res[i]["out_f"], dtype=np.float32).reshape(-1)
            for i in range(N_CORES)
        ]
    )
    return out

